# revision 66
# baseline (speedup 1.0000x reference)
"""BiMPM MatchingLayer kernel for Trainium2, 8 NeuronCores, batch-data-parallel.

Full inputs: p (32,64,600), q (32,64,600), W (8,20,300).
Output: tuple (mv_p, mv_q), each (32,64,160).

Per core: 4 batches x 2 directions (fw: cols 0:300 of p/q, bw: cols 300:600).
All cosine matchings are computed from transposed (h-on-partitions) layouts so
per-row normalizations are per-partition scalars.

Perf structure (cost-model driven):
- Big matmuls (maxpool dl) run in bf16 (1 cyc/row vs fp32's 4).
- The attentive-max product tensor X[h,(s,t)] = srcT[h,t]*C[s,t] is computed
  in bf16 on DVE (2x packed mode), with C replicated across partitions by a
  DMA broadcast-read from a DRAM roundtrip (no PE/PSUM involved).
- max-over-t = 2 levels of pairwise bf16 tensor_max + a final reduce_max,
  alternated between DVE and Pool.
- All normalization/num paths stay fp32, so bf16 rounding only enters via the
  max-product values and the maxpool dot values (~0.5% worst-case, gate 2e-2).
"""

import numpy as np

S, H, L, NB, NCORES = 64, 300, 20, 4, 8
CH = [(0, 128), (128, 256), (256, 300)]
WL = 8 * L

_CACHE = {}


def _bc_mid(bassmod, ap2, n, pos):
    """Insert a stride-0 broadcast dim of count n into a 2D AP's free dims.
    pos=0: (p, f) -> (p, n, f); pos=1: (p, f) -> (p, f, n)."""
    a = list(ap2.ap)
    assert len(a) == 2, a
    if pos == 0:
        new = [a[0], [0, n], a[1]]
    else:
        new = [a[0], a[1], [0, n]]
    return bassmod.AP(tensor=ap2.tensor, offset=ap2.offset, ap=new)


def _build(nb=NB, en=("fu", "mp", "am", "ax"), OFF=120):
    import concourse.bass as bass
    import concourse.tile as tile
    from concourse import bacc, mybir
    from concourse.masks import make_identity
    from contextlib import ExitStack

    f32 = mybir.dt.float32
    bf16 = mybir.dt.float16
    AX = mybir.AxisListType
    OPT = mybir.AluOpType

    nc = bacc.Bacc("TRN2", target_bir_lowering=False, debug=False,
                   enable_asserts=False, num_devices=NCORES)
    p_d = nc.dram_tensor("p", [nb, S, 2 * H], f32, kind="ExternalInput").ap()
    q_d = nc.dram_tensor("q", [nb, S, 2 * H], f32, kind="ExternalInput").ap()
    w_d = nc.dram_tensor("W", [8, L, H], f32, kind="ExternalInput").ap()
    op_d = nc.dram_tensor("op", [nb, S, WL], f32, kind="ExternalOutput").ap()
    oq_d = nc.dram_tensor("oq", [nb, S, WL], f32, kind="ExternalOutput").ap()

    with tile.TileContext(nc) as tc, ExitStack() as ctx:
        const = ctx.enter_context(tc.tile_pool(name="const", bufs=1))
        sb = ctx.enter_context(tc.tile_pool(name="sb", bufs=2))
        sbx = ctx.enter_context(tc.tile_pool(name="sbx", bufs=2))
        sbR = ctx.enter_context(tc.tile_pool(name="sbR", bufs=2))
        sbPQ = ctx.enter_context(tc.tile_pool(name="sbPQ", bufs=3))
        sbX = ctx.enter_context(tc.tile_pool(name="sbX", bufs=4))
        sbX2 = ctx.enter_context(tc.tile_pool(name="sbX2", bufs=2))
        sb3 = ctx.enter_context(tc.tile_pool(name="sb3", bufs=4))
        psT = ctx.enter_context(tc.tile_pool(name="psT", bufs=1, space="PSUM"))
        ps = ctx.enter_context(tc.tile_pool(name="ps", bufs=2, space="PSUM"))
        psN = ctx.enter_context(tc.tile_pool(name="psN", bufs=2, space="PSUM"))
        psdl = ctx.enter_context(tc.tile_pool(name="psdl", bufs=1, space="PSUM"))
        dram = ctx.enter_context(tc.tile_pool(name="dram", bufs=2, space="DRAM"))

        ident = const.tile([128, 128], f32, tag="ident")
        make_identity(nc, ident)
        ones = const.tile([1, 128], f32, tag="ones")
        nc.vector.memset(ones[:], 1.0)
        onesc = const.tile([128, 1], bf16, tag="onesc")
        nc.gpsimd.memset(onesc[:], 1.0)
        onescf = const.tile([128, 1], f32, tag="onescf")
        nc.gpsimd.memset(onescf[:], 1.0)

        # ---- W precompute: VTall[hp, ci, w*L + l] = W[w, l, h0+hp]^2 ----
        vtall = const.tile([128, 3, WL], f32, tag="vtall")
        for w in range(8):
            wt = sb.tile([L, H], f32, tag="wt")
            nc.sync.dma_start(wt[:], w_d[w])
            v2 = sb.tile([L, H], f32, tag="v2")
            nc.vector.tensor_mul(v2[:], wt[:], wt[:])
            for ci, (h0, h1) in enumerate(CH):
                hc = h1 - h0
                pt = ps.tile([128, 192], f32, tag="tC")
                nc.tensor.transpose(pt[:hc, 0:L], v2[:, h0:h1], ident[0:L, 0:L])
                nc.scalar.copy(vtall[:hc, ci, w * L:(w + 1) * L], pt[:hc, 0:L])

        # bf16 copy of W^2 for w=2,3 replicated along t (for mp rhsall builds
        # with packed operands): vrep[hp, ci, d, l, t] = W[2+d, l, h0+hp]^2
        vrep = const.tile([128, 3, 2, L, S], bf16, tag="vrep")
        for ci in range(3):
            for d in range(2):
                w = 2 + d
                src = _bc_mid(bass, vtall[:, ci, w * L:(w + 1) * L], S, 1)
                nc.gpsimd.tensor_copy(vrep[:, ci, d], src)

        def vts(ci, w):
            return vtall[:CH[ci][1] - CH[ci][0], ci, w * L:(w + 1) * L]

        tslot = [0]
        ptbig = psT.tile([128, 384], f32, tag="tT")

        def transpose_to(dst3, dst3bf, src2d, rows):
            """src2d (rows, 300) sbuf -> dst3 (128,3,rows) chunked transpose,
            plus optional bf16 copy into dst3bf. All transposes share one
            persistent PSUM bank, double-slotted by column."""
            for ci, (h0, h1) in enumerate(CH):
                hc = h1 - h0
                off = 192 * (tslot[0] % 2)
                tslot[0] += 1
                nc.tensor.transpose(ptbig[:hc, off:off + rows], src2d[:, h0:h1],
                                    ident[0:rows, 0:rows])
                nc.scalar.copy(dst3[:hc, ci, :], ptbig[:hc, off:off + rows])
                if dst3bf is not None:
                    nc.gpsimd.tensor_copy(dst3bf[:hc, ci, :], dst3[:hc, ci, :])

        def flat3(t3, hc, ci):
            """(128,3,A,B) tile -> (hc, A*B) 2D AP for chunk ci."""
            ap = t3[:hc, ci]
            a = list(ap.ap)
            n = 1
            for st, ct in a[1:]:
                n *= ct
            return bass.AP(tensor=ap.tensor, offset=ap.offset, ap=[a[0], [1, n]])

        def mp_tail(numps, y_t, w, invnAll, side_out, sgn=None, clamp=True):
            """Common mp_cos tail: given num (64,L) psum and transposed y (128,3,S),
            compute den from y^2 @ V_w, combine, write to side_out slice."""
            y2 = sb.tile([128, 3, S], f32, tag="y2")
            nc.vector.tensor_mul(y2[:], y_t[:], y_t[:])
            denps = psN.tile([128, 192], f32, tag="tN")
            for ci, (h0, h1) in enumerate(CH):
                hc = h1 - h0
                nc.tensor.matmul(denps[0:S, 0:L], y2[:hc, ci, :], vts(ci, w),
                                 start=(ci == 0), stop=(ci == 2))
            ny = sb.tile([S, L], f32, tag="ny")
            nc.scalar.sqrt(ny[:], denps[0:S, 0:L])
            invy = sb.tile([S, L], f32, tag="invy")
            scr = sb.tile([S, L], f32, tag="scrL")
            nc.vector.reciprocal_approx_accurate(invy[:], ny[:], scr[:])
            c1 = sb.tile([S, L], f32, tag="c1")
            nc.vector.tensor_mul(c1[:], invnAll[:, w * L:(w + 1) * L], invy[:])
            if clamp:
                c2 = sb.tile([S, L], f32, tag="c2")
                nc.vector.tensor_scalar_min(c2[:], c1[:], 1e8)
                c1 = c2
            if sgn is not None:
                nc.vector.scalar_tensor_tensor(
                    out=side_out[:, w * L:(w + 1) * L], in0=numps[0:S, 0:L],
                    scalar=sgn[:], in1=c1[:], op0=OPT.mult, op1=OPT.mult)
            else:
                nc.vector.tensor_mul(side_out[:, w * L:(w + 1) * L],
                                     numps[0:S, 0:L], c1[:])

        red_rr = [0]  # round-robin DVE/Pool for reduce_max
        sbO = ctx.enter_context(tc.tile_pool(name="sbO", bufs=8))

        outs_by_b = {}
        for d in range(2):
            for b in range(nb):
                if d == 0:
                    opt = sbO.tile([S, WL], f32, tag="OP")
                    oqt = sbO.tile([S, WL], f32, tag="OQ")
                    outs_by_b[b] = (opt, oqt)
                    if len(en) < 4:
                        nc.gpsimd.memset(opt[:], 0.0)
                        nc.gpsimd.memset(oqt[:], 0.0)
                else:
                    opt, oqt = outs_by_b[b]
                c0 = d * H
                fe = tc.high_priority(offset=OFF)
                fe.__enter__()
                P = sbPQ.tile([S, H], f32, tag="P")
                nc.sync.dma_start(P[:], p_d[b, :, c0:c0 + H])
                Q = sbPQ.tile([S, H], f32, tag="Q")
                nc.sync.dma_start(Q[:], q_d[b, :, c0:c0 + H])

                PT = sb3.tile([128, 3, S], f32, tag="PT")
                PTb = sb3.tile([128, 3, S], bf16, tag="PTb")
                transpose_to(PT, PTb, P, S)
                QT = sb3.tile([128, 3, S], f32, tag="QT")
                QTb = sb3.tile([128, 3, S], bf16, tag="QTb")
                transpose_to(QT, QTb, Q, S)
                PT2 = sb3.tile([128, 3, S], f32, tag="PT2")
                nc.vector.tensor_mul(PT2[:], PT[:], PT[:])
                QT2 = sb3.tile([128, 3, S], f32, tag="QT2")
                nc.vector.tensor_mul(QT2[:], QT[:], QT[:])

                # ---- plain row norms + normalized cosine matrix ----
                scr300 = sb.tile([S, H], f32, tag="scr300")
                nsq = sb.tile([S, 1], f32, tag="nsqP")
                nc.vector.tensor_mul(scr300[:], P[:], P[:])
                nc.vector.reduce_sum(out=nsq[:], in_=scr300[:], axis=AX.X)
                invnP = sb.tile([S, 1], f32, tag="invnP")
                nP = sb.tile([S, 1], f32, tag="nP")
                nc.scalar.sqrt(nP[:], nsq[:])
                nc.vector.reciprocal(invnP[:], nP[:])
                nsqQ = sb.tile([S, 1], f32, tag="nsqQ")
                scr300b = sb.tile([S, H], f32, tag="scr300b")
                nc.vector.tensor_mul(scr300b[:], Q[:], Q[:])
                nc.vector.reduce_sum(out=nsqQ[:], in_=scr300b[:], axis=AX.X)
                invnQ = sb.tile([S, 1], f32, tag="invnQ")
                nQ = sb.tile([S, 1], f32, tag="nQ")
                nc.scalar.sqrt(nQ[:], nsqQ[:])
                nc.vector.reciprocal(invnQ[:], nQ[:])

                Qn = sb.tile([S, H], f32, tag="Qn")
                nc.vector.tensor_scalar_mul(Qn[:], Q[:], invnQ[:])
                QnT = sb3.tile([128, 3, S], f32, tag="QnT")
                transpose_to(QnT, None, Qn, S)

                cut = ps.tile([128, 192], f32, tag="tC")
                for ci, (h0, h1) in enumerate(CH):
                    hc = h1 - h0
                    nc.tensor.matmul(cut[0:S, 0:S], QnT[:hc, ci, :], PT[:hc, ci, :],
                                     start=(ci == 0), stop=(ci == 2))
                cut_sb = sb.tile([S, S], f32, tag="cut_sb")
                nc.scalar.copy(cut_sb[:], cut[0:S, 0:S])
                cu = ps.tile([128, 192], f32, tag="tC")
                nc.tensor.transpose(cu[0:S, 0:S], cut_sb[:], ident[0:S, 0:S])
                Cs = sb3.tile([S, S], f32, tag="Cs")
                nc.vector.tensor_scalar_mul(Cs[:], cu[0:S, 0:S], invnP[:])
                ctp = ps.tile([128, 192], f32, tag="tC")
                nc.tensor.transpose(ctp[0:S, 0:S], Cs[:], ident[0:S, 0:S])
                Ct = sb3.tile([S, S], f32, tag="Ct")
                nc.scalar.copy(Ct[:], ctp[0:S, 0:S])

                # bf16 copies of the cosine matrices (for ax replication)
                Csb = sb3.tile([S, S], bf16, tag="Csb")
                nc.gpsimd.tensor_copy(Csb[:], Cs[:])
                Ctb = sb3.tile([S, S], bf16, tag="Ctb")
                nc.gpsimd.tensor_copy(Ctb[:], Ct[:])

                # ---- weighted norms, all 8 perspectives: (64, 160) ----
                p2v = ps.tile([128, 192], f32, tag="tC")
                for ci, (h0, h1) in enumerate(CH):
                    hc = h1 - h0
                    nc.tensor.matmul(p2v[0:S, 0:WL], PT2[:hc, ci, :], vtall[:hc, ci, :],
                                     start=(ci == 0), stop=(ci == 2))
                invnpAll = sb.tile([S, WL], f32, tag="invnpAll")
                npw = sb.tile([S, WL], f32, tag="npw")
                nc.scalar.sqrt(npw[:], p2v[0:S, 0:WL])
                scrW = sb.tile([S, WL], f32, tag="scrW")
                nc.vector.reciprocal_approx_accurate(invnpAll[:], npw[:], scrW[:])
                q2v = ps.tile([128, 192], f32, tag="tC")
                for ci, (h0, h1) in enumerate(CH):
                    hc = h1 - h0
                    nc.tensor.matmul(q2v[0:S, 0:WL], QT2[:hc, ci, :], vtall[:hc, ci, :],
                                     start=(ci == 0), stop=(ci == 2))
                invnqAll = sb.tile([S, WL], f32, tag="invnqAll")
                nqw = sb.tile([S, WL], f32, tag="nqw")
                nc.scalar.sqrt(nqw[:], q2v[0:S, 0:WL])
                scrW2 = sb.tile([S, WL], f32, tag="scrW2")
                nc.vector.reciprocal_approx_accurate(invnqAll[:], nqw[:], scrW2[:])

                # -- mp prep: transposed weighted inverse norms + DMA replicate --
                reps = {}
                if "mp" in en:
                    wmp = 2 + d
                    for (nm, xT2) in (("q", QT2),):
                        nvt = ps.tile([128, 192], f32, tag="tC")
                        for ci, (h0, h1) in enumerate(CH):
                            hc = h1 - h0
                            nc.tensor.matmul(nvt[0:L, 0:S], vts16(ci, wmp), xT2[:hc, ci, :],
                                             start=(ci == 0), stop=(ci == 2))
                        nT = sb.tile([L, S], f32, tag="nT")
                        nc.scalar.sqrt(nT[:], nvt[0:L, 0:S])
                        invT = sb.tile([L, S], f32, tag="invT")
                        scrT = sb.tile([L, S], f32, tag="scrT")
                        nc.vector.reciprocal_approx_accurate(invT[:], nT[:], scrT[:])
                        invT16 = sb.tile([L, S], bf16, tag="invT16")
                        nc.gpsimd.tensor_copy(invT16[:], invT[:])
                        scrd = dram.tile([L, S], bf16, tag="nTd")
                        nc.scalar.dma_start(scrd[:], invT16[:])
                        repsb = sb.tile([S, L * S], bf16, tag="invR" + nm)
                        srcp = bass.AP(tensor=scrd.tensor, offset=scrd.offset,
                                       ap=[[0, S], [1, L * S]])
                        nc.scalar.dma_start(repsb[:], srcp)
                        reps[nm] = repsb

                # -- ax prep: DRAM roundtrip bf16 cosine -> 128-partition reps --
                repc = {}
                if "ax" in en:
                    cds = {}
                    for (nm, cb) in (("s", Csb), ("t", Ctb)):
                        cd = dram.tile([S, S], bf16, tag="cd" + nm)
                        nc.scalar.dma_start(cd[:], cb[:])
                        cds[nm] = cd
                        rep = sbR.tile([128, S * S], bf16, tag="rep" + nm)
                        nc.scalar.dma_start(rep[:], bass.AP(
                            tensor=cd.tensor, offset=cd.offset,
                            ap=[[0, 128], [1, S * S]]))
                        repc[nm] = rep
                    # fused-ci2 operands: repmix = rep(Cs)@0:44 | rep(Ct)@44:88,
                    # stackQP = QTb-ci2@0:44 | PTb-ci2@44:88 (partition shift via DMA)
                    repmix = sbR.tile([88, S * S], bf16, tag="repmix")
                    nc.scalar.dma_start(repmix[0:44, :], bass.AP(
                        tensor=cds["s"].tensor, offset=cds["s"].offset,
                        ap=[[0, 44], [1, S * S]]))
                    nc.scalar.dma_start(repmix[44:88, :], bass.AP(
                        tensor=cds["t"].tensor, offset=cds["t"].offset,
                        ap=[[0, 44], [1, S * S]]))
                    stackQP = sb3.tile([88, S], bf16, tag="stackQP")
                    nc.gpsimd.tensor_copy(stackQP[0:44, :], QTb[0:44, 2, :])
                    nc.scalar.dma_start(stackQP[44:88, :], PTb[0:44, 2, :])
                fe.__exit__(None, None, None)

                # ============ FULL matching (w = d) ============
                if "fu" in en:
                    w = d
                    tidx = S - 1 if d == 0 else 0
                    for (side_out, xT, yT, yT2, invnAll) in (
                            (opt, PT, QT, QT2, invnpAll),
                            (oqt, QT, PT, PT2, invnqAll)):
                        g = sb.tile([128, 3, S], f32, tag="gf")
                        for ci in range(3):
                            nc.vector.tensor_scalar_mul(
                                g[:, ci, :], xT[:, ci, :], yT[:, ci, tidx:tidx + 1])
                        nums = psN.tile([128, 192], f32, tag="tN")
                        for ci, (h0, h1) in enumerate(CH):
                            hc = h1 - h0
                            nc.tensor.matmul(nums[0:S, 0:L], g[:hc, ci, :], vts(ci, w),
                                             start=(ci == 0), stop=(ci == 2))
                        ql = psN.tile([128, 192], f32, tag="tN")
                        for ci, (h0, h1) in enumerate(CH):
                            hc = h1 - h0
                            nc.tensor.matmul(ql[0:1, 0:L], yT2[:hc, ci, tidx:tidx + 1],
                                             vts(ci, w), start=(ci == 0), stop=(ci == 2))
                        qln = sb.tile([1, L], f32, tag="qln")
                        nc.scalar.sqrt(qln[:], ql[0:1, 0:L])
                        invql = sb.tile([1, L], f32, tag="invql")
                        scr1 = sb.tile([1, L], f32, tag="scr1")
                        nc.vector.reciprocal_approx_accurate(invql[:], qln[:], scr1[:])
                        qlr = psN.tile([128, 192], f32, tag="tN")
                        nc.tensor.matmul(qlr[0:S, 0:L], ones[0:1, 0:S], invql[0:1, :],
                                         start=True, stop=True)
                        c1 = sb.tile([S, L], f32, tag="fc1")
                        nc.vector.tensor_mul(c1[:], invnAll[:, w * L:(w + 1) * L],
                                             qlr[0:S, 0:L])
                        c2 = sb.tile([S, L], f32, tag="fc2")
                        nc.vector.tensor_scalar_min(c2[:], c1[:], 1e8)
                        nc.vector.tensor_mul(side_out[:, w * L:(w + 1) * L],
                                             nums[0:S, 0:L], c2[:])

                # ============ MAXPOOL matching (w = 2 + d) ============
                if "mp" in en:
                    w = 2 + d
                    # transposed weighted inverse norms for the "inner" side,
                    # then DRAM-roundtrip flatten + broadcast to 64 partitions.
                    reps = {}
                    for (nm, xT2) in (("q", QT2), ("p", PT2)):
                        nvt = ps.tile([128, 192], f32, tag="tC")
                        for ci, (h0, h1) in enumerate(CH):
                            hc = h1 - h0
                            nc.tensor.matmul(nvt[0:L, 0:S], vts(ci, w), xT2[:hc, ci, :],
                                             start=(ci == 0), stop=(ci == 2))
                        nT = sb.tile([L, S], f32, tag="nT")
                        nc.scalar.sqrt(nT[:], nvt[0:L, 0:S])
                        invT = sb.tile([L, S], f32, tag="invT")
                        scrT = sb.tile([L, S], f32, tag="scrT")
                        nc.vector.reciprocal_approx_accurate(invT[:], nT[:], scrT[:])
                        scrd = dram.tile([L, S], f32, tag="nTd")
                        nc.sync.dma_start(scrd[:], invT[:])
                        repsb = sb.tile([S, L * S], f32, tag="invR" + nm)
                        src = bass.AP(tensor=scrd.tensor, offset=scrd.offset,
                                      ap=[[0, S], [1, L * S]])
                        nc.sync.dma_start(repsb[:], src)
                        reps[nm] = repsb

                    for (side_out, statTb, rhs_srcTb, invR, invnAll) in (
                            (opt, PTb, QTb, reps["q"], invnpAll),
                            (oqt, QTb, PTb, reps["p"], invnqAll)):
                        rhsall = sbx.tile([128, 3, L, S], bf16, tag="rhsall")
                        for ci in range(3):
                            in0 = _bc_mid(bass, rhs_srcTb[:, ci, :], L, 0)
                            nc.vector.tensor_mul(rhsall[:, ci], in0, vrep[:, ci, d])
                        dl = psdl.tile([S, L * S], f32, tag="dl")
                        for ci, (h0, h1) in enumerate(CH):
                            hc = h1 - h0
                            lt = statTb[:hc, ci, :]
                            rh = flat3(rhsall, hc, ci)
                            for (n0, n1) in ((0, 512), (512, 1024), (1024, 1280)):
                                nc.tensor.matmul(dl[:, n0:n1], lt, rh[:, n0:n1],
                                                 start=(ci == 0), stop=(ci == 2))
                        dsc = sb.tile([S, L, S], bf16, tag="dsc")
                        dl3 = bass.AP(tensor=dl.tensor, offset=dl.offset,
                                      ap=[list(dl.ap[0]), [S, L], [1, S]])
                        invR3 = bass.AP(tensor=invR.tensor, offset=invR.offset,
                                        ap=[list(invR.ap[0]), [S, L], [1, S]])
                        nc.vector.tensor_mul(dsc[:], dl3, invR3)
                        # 2-level fp16 max tree + final reduce over 16
                        eng = nc.vector if red_rr[0] % 2 == 0 else nc.gpsimd
                        red_rr[0] += 1
                        eng.tensor_max(dsc[:, :, 0:32], dsc[:, :, 0:32],
                                       dsc[:, :, 32:64])
                        nc.vector.tensor_max(dsc[:, :, 0:16], dsc[:, :, 0:16],
                                             dsc[:, :, 16:32])
                        mx = sb.tile([S, L], f32, tag="mx")
                        dsc3 = bass.AP(tensor=dsc.tensor, offset=dsc.offset,
                                       ap=[list(dsc.ap[0]), [S * 2, L], [1, 16]])
                        nc.vector.reduce_max(out=mx[:], in_=dsc3, axis=AX.X)
                        nc.vector.tensor_mul(side_out[:, w * L:(w + 1) * L], mx[:],
                                             invnAll[:, w * L:(w + 1) * L])

                # ============ ATTENTIVE-MEAN matching (w = 4 + d) ============
                if "am" in en:
                    w = 4 + d
                    rs = sb.tile([S, 1], f32, tag="rs")
                    nc.vector.reduce_sum(out=rs[:], in_=Cs[:], axis=AX.X)
                    sgr = sb.tile([S, 1], f32, tag="sgr")
                    nc.scalar.sign(sgr[:], rs[:])
                    cs_ = sb.tile([S, 1], f32, tag="cs_")
                    nc.vector.reduce_sum(out=cs_[:], in_=Ct[:], axis=AX.X)
                    sgc = sb.tile([S, 1], f32, tag="sgc")
                    nc.scalar.sign(sgc[:], cs_[:])

                    for (side_out, nat, cmat, statTb, invnAll, sg) in (
                            (opt, Q, Ct, PTb, invnpAll, sgr),
                            (oqt, P, Cs, QTb, invnqAll, sgc)):
                        yvu = psN.tile([128, 192], f32, tag="tN")
                        for ci, (h0, h1) in enumerate(CH):
                            hc = h1 - h0
                            nc.tensor.matmul(yvu[:hc, ci * S:(ci + 1) * S],
                                             nat[:, h0:h1], cmat[:],
                                             start=True, stop=True)
                        yvs = sb.tile([128, 3, S], f32, tag="yvs")
                        nc.scalar.copy(yvs[:], bass.AP(
                            tensor=yvu.tensor, offset=yvu.offset,
                            ap=[list(yvu.ap[0]), [S, 3], [1, S]]))
                        g = sb.tile([128, 3, S], f32, tag="gam")
                        nc.vector.tensor_mul(g[:], statT[:], yvs[:])
                        nums = psN.tile([128, 192], f32, tag="tN")
                        for ci, (h0, h1) in enumerate(CH):
                            hc = h1 - h0
                            nc.tensor.matmul(nums[0:S, 0:L], g[:hc, ci, :], vts(ci, w),
                                             start=(ci == 0), stop=(ci == 2))
                        mp_tail(nums, yvs, w, invnAll, side_out, sgn=sg, clamp=True)

                # ============ ATTENTIVE-MAX matching (w = 6 + d) ============
                if "ax" in en:
                    w = 6 + d
                    res = {}
                    for (nm, rep, srcTb) in (("q", repc["s"], QTb),
                                             ("p", repc["t"], PTb)):
                        ymaxT = sb3.tile([128, 3, S], f32, tag="ymaxT" + nm)
                        rep3 = bass.AP(tensor=rep.tensor, offset=rep.offset,
                                       ap=[list(rep.ap[0]), [S, S], [1, S]])
                        for ci in range(3):
                            X = sbX.tile([128, S, S], bf16, tag="X")
                            in0 = _bc_mid(bass, srcTb[:, ci, :], S, 0)
                            nc.vector.tensor_mul(X[:], in0, rep3)
                            eng = nc.gpsimd if red_rr[0] % 3 == 2 else nc.vector
                            red_rr[0] += 1
                            eng.tensor_max(X[:, :, 0:32], X[:, :, 0:32],
                                           X[:, :, 32:64])
                            nc.vector.tensor_max(X[:, :, 0:16], X[:, :, 0:16],
                                                 X[:, :, 16:32])
                            x16 = bass.AP(tensor=X.tensor, offset=X.offset,
                                          ap=[list(X.ap[0]), [S, S], [1, 16]])
                            nc.vector.reduce_max(out=ymaxT[:, ci, :], in_=x16, axis=AX.X)
                        res[nm] = ymaxT

                    for (side_out, statT, ymaxT, invnAll) in (
                            (opt, PT, res["q"], invnpAll),
                            (oqt, QT, res["p"], invnqAll)):
                        g = sb.tile([128, 3, S], f32, tag="gax")
                        nc.vector.tensor_mul(g[:], statT[:], ymaxT[:])
                        nums = psN.tile([128, 192], f32, tag="tN")
                        for ci, (h0, h1) in enumerate(CH):
                            hc = h1 - h0
                            nc.tensor.matmul(nums[0:S, 0:L], g[:hc, ci, :], vts(ci, w),
                                             start=(ci == 0), stop=(ci == 2))
                        mp_tail(nums, ymaxT, w, invnAll, side_out, sgn=None, clamp=True)

                if d == 1:
                    nc.scalar.dma_start(op_d[b], opt[:])
                    nc.scalar.dma_start(oq_d[b], oqt[:])

    nc.compile()
    return nc


def _get_nc(nb=NB, en=("fu", "mp", "am", "ax")):
    key = (nb, tuple(en))
    if key not in _CACHE:
        _CACHE[key] = _build(nb, en)
    return _CACHE[key]


def _run(p, q, W, nb=NB, en=("fu", "mp", "am", "ax"), trace=False):
    from concourse.bass_utils import run_bass_kernel_spmd
    nc = _get_nc(nb, en)
    B = p.shape[0]
    ncores = B // nb
    assert ncores == NCORES and B == nb * NCORES
    in_maps = []
    for c in range(NCORES):
        in_maps.append({
            "p": np.ascontiguousarray(p[c * nb:(c + 1) * nb]),
            "q": np.ascontiguousarray(q[c * nb:(c + 1) * nb]),
            "W": np.ascontiguousarray(W),
        })
    r = run_bass_kernel_spmd(nc, in_maps, core_ids=list(range(NCORES)), trace=trace)
    if trace:
        print("HW exec time:", r.exec_time_ns, "ns")
        print("trace:", r.instructions_and_trace[1] if r.instructions_and_trace else None)
    mv_p = np.concatenate([r.results[c]["op"] for c in range(NCORES)], axis=0)
    mv_q = np.concatenate([r.results[c]["oq"] for c in range(NCORES)], axis=0)
    return mv_p, mv_q


def kernel(p, q, W):
    p = np.asarray(p, dtype=np.float32)
    q = np.asarray(q, dtype=np.float32)
    W = np.asarray(W, dtype=np.float32)
    return _run(p, q, W)
